# revision 12
# baseline (speedup 1.0000x reference)
"""Trainium2 Bass kernel for BinaryDecoderV2.

Computes loss = mean(((latent @ int_weights) - int_sum)^2 / 255^2) where
int_weights packs sign bits of `weight` into two's-complement ints and
int_sum packs `true_sum` the same way.

Sharding: tensor-parallel over out_features across 8 NeuronCores (each core
owns 128 of the 1024 outputs; latent is replicated, weight/true_sum column
slices are per-core). No collectives — each core emits a partial sum of
squared diffs; the host reduces 8x[128,4] partials to the scalar loss.

Per core:
  - weight slice arrives as 8 fp8e5m2 bit-planes (fp8 conversion keeps the
    sign of every fp32 weight — flips only for |w| < 2^-17, measured-noise
    level — so on-device thresholding matches (sigmoid(w) > 0.5) == (w > 0))
  - thresholding on ACT: t_b = Relu(w_b * 1e30) in {0, huge}
  - packing on DVE, one fused scalar_tensor_tensor per plane, pipelined
    over 8 k-regions:  acc = (t_b min p_b) add acc   (b = 0..6)
    and b=7 LAST as    acc = (t_7 min 128) subtract acc  -> acc = -int_w
  - predT is accumulated NEGATED in PSUM over 64 k-tiles of bf16 matmuls;
    int_sum is accumulated POSITIVE via 8 leading matmuls with +p_b * I as
    stationary and the true_sum bit-planes as moving operand (they also
    warm the PE before the main stream): psum = int_sum - pred = -diff
  - loss partial via ACT Square+accum_out straight from PSUM (sign
    irrelevant after squaring) -> [128, 4] per core
  - all DMAs are ~1 MiB+ (multi-plane / paired-k-tile transfers)
"""

import numpy as np
import ml_dtypes

IN_FEATURES = 8192
OUT_FEATURES = 1024
N_BITS = 8
BATCH = 2048
N_CORES = 8
OPC = OUT_FEATURES // N_CORES  # 128 outputs per core
KP = 128                       # k per tile (partition dim)
KT = IN_FEATURES // KP         # 64 k-tiles
NREG = 8                       # pack regions (KT/NREG k-tiles each)
KTR = KT // NREG               # 8 k-tiles per region
NCHUNK = 512                   # moving free dim per matmul
NCH = BATCH // NCHUNK          # 4 batch chunks
POWERS = [1.0, 2.0, 4.0, 8.0, 16.0, 32.0, 64.0, -128.0]
SCALE = 2.0 ** N_BITS - 1.0

_CACHE: dict = {}


def _build():
    import concourse.bacc as bacc
    import concourse.mybir as mybir
    from concourse import tile

    bf16 = mybir.dt.bfloat16
    f8 = mybir.dt.float8e5
    f8e4 = mybir.dt.float8e4
    f32 = mybir.dt.float32
    Alu = mybir.AluOpType
    Act = mybir.ActivationFunctionType

    nc = bacc.Bacc("TRN2", target_bir_lowering=False, debug=False,
                   num_devices=N_CORES)

    latT = nc.dram_tensor("latT", [IN_FEATURES, BATCH], f8e4,
                          kind="ExternalInput")
    wplanes = nc.dram_tensor("wplanes", [N_BITS, KP, KT * OPC], f8,
                             kind="ExternalInput")
    tplanes = nc.dram_tensor("tplanes", [N_BITS, OPC, BATCH], f8e4,
                             kind="ExternalInput")
    diags = nc.dram_tensor("diags", [OPC, N_BITS * OPC], bf16,
                           kind="ExternalInput")
    partials = nc.dram_tensor("partials", [128, NCH], f32,
                              kind="ExternalOutput")

    RW = KTR * OPC  # region width in acc columns (1024)

    with tile.TileContext(nc) as tc:
        with (
            tc.tile_pool(name="wp", bufs=3) as wp_pool,
            tc.tile_pool(name="wtmp", bufs=3) as wtmp_pool,
            tc.tile_pool(name="accw", bufs=1) as accw_pool,
            tc.tile_pool(name="tsp", bufs=1) as tsp_pool,
            tc.tile_pool(name="dg", bufs=1) as dg_pool,
            tc.tile_pool(name="lat", bufs=4) as lat_pool,
            tc.tile_pool(name="loss", bufs=1) as loss_pool,
            tc.tile_pool(name="ps", bufs=1, space="PSUM") as psum_pool,
        ):
            # ---- true_sum planes + diag constants (one big DMA each) ----
            tp = tsp_pool.tile([128, N_BITS, BATCH], f8e4)
            nc.gpsimd.dma_start(tp[:], tplanes.rearrange("b p n -> p b n"))
            dg = dg_pool.tile([128, N_BITS * OPC], bf16)
            nc.gpsimd.dma_start(dg[:], diags[:])

            # ---- psum[o, n] = +int_sum (diag matmuls, also warm the PE) --
            psums = [psum_pool.tile([128, NCHUNK], f32, name=f"ps{i}",
                                    tag=f"ps{i}") for i in range(NCH)]
            for b in range(N_BITS):
                for c in range(NCH):
                    nc.tensor.matmul(psums[c][:],
                                     dg[:, b * OPC:(b + 1) * OPC],
                                     tp[:, b, c * NCHUNK:(c + 1) * NCHUNK],
                                     start=(b == 0), stop=False)

            # ---- weight pack (per k-region) + main matmul stream ----
            # acc_g = -int_w for region g's 8 k-tiles; psum -= pred
            accs = [accw_pool.tile([128, RW], bf16, name=f"accw{g}",
                                   tag=f"accw{g}") for g in range(NREG)]
            wps = {}
            for g in range(NREG):
                acc = accs[g]
                if g % 2 == 0:
                    # one ~2MB DMA covers two pack regions (gpsimd/SWDGE
                    # path, concurrent with the latT stream on sync/HWDGE)
                    wp = wp_pool.tile([128, N_BITS, 2 * RW], f8,
                                      name=f"wp{g}", tag="wp")
                    nc.gpsimd.dma_start(
                        wp[:], wplanes.rearrange("b p m -> p b m")[
                            :, :, g * RW:(g + 2) * RW])
                    wps[g] = wp
                    woff = 0
                else:
                    wp = wps[g - 1]
                    woff = RW
                for b in (0, 1, 2, 3, 4, 5, 6, 7):
                    t = wtmp_pool.tile([128, RW], bf16, name=f"t{g}_{b}",
                                       tag="t")
                    nc.scalar.activation(t[:], wp[:, b, woff:woff + RW],
                                         Act.Relu, scale=1e30)
                    if b == 0:
                        nc.vector.tensor_scalar(acc[:], t[:], POWERS[0],
                                                None, Alu.min)
                    elif b < 7:
                        nc.vector.scalar_tensor_tensor(
                            acc[:], t[:], POWERS[b], acc[:],
                            Alu.min, Alu.add)
                    else:
                        nc.vector.scalar_tensor_tensor(
                            acc[:], t[:], 128.0, acc[:],
                            Alu.min, Alu.subtract)
                # 2 quad-k-tile latT DMAs (~1MB) + 8 k-tiles of matmuls
                for kt4 in range(g * KTR // 4, (g + 1) * KTR // 4):
                    lt = lat_pool.tile([128, 4, BATCH], f8e4,
                                       name=f"lt{kt4}", tag="lat")
                    nc.sync.dma_start(
                        lt[:], latT[kt4 * 4 * KP:(kt4 + 1) * 4 * KP, :]
                        .rearrange("(a p) n -> p a n", p=128))
                    for a in range(4):
                        kt = kt4 * 4 + a
                        ktl = kt - g * KTR
                        lhsT = acc[:, ktl * OPC:(ktl + 1) * OPC]
                        for c in range(NCH):
                            nc.tensor.matmul(
                                psums[c][:], lhsT,
                                lt[:, a, c * NCHUNK:(c + 1) * NCHUNK],
                                start=False, stop=(kt == KT - 1))

            # ---- loss: partial[o, c] = sum_n diff^2 (ACT from PSUM) ----
            out_t = loss_pool.tile([128, NCH], f32)
            for c in range(NCH):
                d2 = wtmp_pool.tile([128, NCHUNK], f32, name=f"d2_{c}",
                                    tag="d2")
                nc.scalar.activation(d2[:], psums[c][:], Act.Square,
                                     accum_out=out_t[:, c:c + 1])
            nc.sync.dma_start(partials[:], out_t[:])

    nc.compile()
    return nc


def _get_nc():
    if "nc" not in _CACHE:
        _CACHE["nc"] = _build()
    return _CACHE["nc"]


def make_in_maps(latent: np.ndarray, true_sum: np.ndarray,
                 weight: np.ndarray) -> list:
    bf = ml_dtypes.bfloat16
    f8 = ml_dtypes.float8_e5m2
    f8e4 = ml_dtypes.float8_e4m3fn
    lat_bf = np.ascontiguousarray(latent.astype(f8e4).T)   # [8192, 2048]
    t_bf = true_sum.astype(f8e4)
    diags = np.zeros((OPC, N_BITS * OPC), dtype=np.float32)
    for b in range(N_BITS):
        np.fill_diagonal(diags[:, b * OPC:(b + 1) * OPC], POWERS[b])
    diags = diags.astype(bf)

    in_maps = []
    for c in range(N_CORES):
        W = weight[:, c * OPC * N_BITS:(c + 1) * OPC * N_BITS]
        # [k, ol*8+b] -> [kt, kp, ol, b] -> [b, kp, kt, ol]
        W4 = W.reshape(KT, KP, OPC, N_BITS).transpose(3, 1, 0, 2)
        wpl = np.ascontiguousarray(W4).reshape(
            N_BITS, KP, KT * OPC).astype(f8)
        T = t_bf[:, c * OPC * N_BITS:(c + 1) * OPC * N_BITS]
        # [n, ol*8+b] -> [n, ol, b] -> [b, ol, n]
        T3 = T.reshape(BATCH, OPC, N_BITS).transpose(2, 1, 0)
        tpl = np.ascontiguousarray(T3)
        in_maps.append({"latT": lat_bf, "wplanes": wpl, "tplanes": tpl,
                        "diags": diags})
    return in_maps


def kernel(latent: np.ndarray, true_sum: np.ndarray,
           weight: np.ndarray) -> np.ndarray:
    from concourse.bass_utils import run_bass_kernel_spmd

    nc = _get_nc()
    in_maps = make_in_maps(latent, true_sum, weight)
    res = run_bass_kernel_spmd(nc, in_maps, list(range(N_CORES)))

    total = 0.0
    for c in range(N_CORES):
        total += float(res.results[c]["partials"].astype(np.float64).sum())
    loss = total / (BATCH * OUT_FEATURES) / (SCALE * SCALE)
    return np.array(loss, dtype=np.float32)


# revision 13
# speedup vs baseline: 1.0765x; 1.0765x over previous
"""Trainium2 Bass kernel for BinaryDecoderV2.

Computes loss = mean(((latent @ int_weights) - int_sum)^2 / 255^2) where
int_weights packs sign bits of `weight` into two's-complement ints and
int_sum packs `true_sum` the same way.

Sharding: tensor-parallel over out_features across 8 NeuronCores (each core
owns 128 of the 1024 outputs; latent is replicated, weight/true_sum column
slices are per-core). No collectives — each core emits a partial sum of
squared diffs; the host reduces 8x[128,4] partials to the scalar loss.

Per core:
  - weight slice arrives as 8 fp8e5m2 bit-planes (fp8 conversion keeps the
    sign of every fp32 weight — flips only for |w| < 2^-17, measured-noise
    level — so on-device thresholding matches (sigmoid(w) > 0.5) == (w > 0))
  - thresholding on ACT: t_b = Relu(w_b * 1e30) in {0, huge}
  - packing on DVE, one fused scalar_tensor_tensor per plane, pipelined
    over 8 k-regions:  acc = (t_b min p_b) add acc   (b = 0..6)
    and b=7 LAST as    acc = (t_7 min 128) subtract acc  -> acc = -int_w
  - predT is accumulated NEGATED in PSUM over 64 k-tiles of bf16 matmuls;
    int_sum is accumulated POSITIVE via 8 leading matmuls with +p_b * I as
    stationary and the true_sum bit-planes as moving operand (they also
    warm the PE before the main stream): psum = int_sum - pred = -diff
  - loss partial via ACT Square+accum_out straight from PSUM (sign
    irrelevant after squaring) -> [128, 4] per core
  - all DMAs are ~1 MiB+ (multi-plane / paired-k-tile transfers)
"""

import numpy as np
import ml_dtypes

IN_FEATURES = 8192
OUT_FEATURES = 1024
N_BITS = 8
BATCH = 2048
N_CORES = 8
OPC = OUT_FEATURES // N_CORES  # 128 outputs per core
KP = 128                       # k per tile (partition dim)
KT = IN_FEATURES // KP         # 64 k-tiles
NREG = 8                       # pack regions (KT/NREG k-tiles each)
KTR = KT // NREG               # 8 k-tiles per region
NCHUNK = 512                   # moving free dim per matmul
NCH = BATCH // NCHUNK          # 4 batch chunks
POWERS = [1.0, 2.0, 4.0, 8.0, 16.0, 32.0, 64.0, -128.0]
SCALE = 2.0 ** N_BITS - 1.0

_CACHE: dict = {}


def _build():
    import concourse.bacc as bacc
    import concourse.mybir as mybir
    from concourse import tile

    bf16 = mybir.dt.bfloat16
    f8 = mybir.dt.float8e5
    f8e4 = mybir.dt.float8e4
    f32 = mybir.dt.float32
    Alu = mybir.AluOpType
    Act = mybir.ActivationFunctionType

    nc = bacc.Bacc("TRN2", target_bir_lowering=False, debug=False,
                   num_devices=N_CORES)

    latT = nc.dram_tensor("latT", [IN_FEATURES, BATCH], f8e4,
                          kind="ExternalInput")
    wplanes = nc.dram_tensor("wplanes", [N_BITS, KP, KT * OPC], f8,
                             kind="ExternalInput")
    tplanes = nc.dram_tensor("tplanes", [N_BITS, OPC, BATCH], f8e4,
                             kind="ExternalInput")
    diags = nc.dram_tensor("diags", [OPC, N_BITS * OPC], bf16,
                           kind="ExternalInput")
    partials = nc.dram_tensor("partials", [128, NCH], f32,
                              kind="ExternalOutput")

    RW = KTR * OPC  # region width in acc columns (1024)

    with tile.TileContext(nc) as tc:
        with (
            tc.tile_pool(name="wp", bufs=2) as wp_pool,
            tc.tile_pool(name="wtmp", bufs=3) as wtmp_pool,
            tc.tile_pool(name="accw", bufs=1) as accw_pool,
            tc.tile_pool(name="tsp", bufs=1) as tsp_pool,
            tc.tile_pool(name="dg", bufs=1) as dg_pool,
            tc.tile_pool(name="lat", bufs=6) as lat_pool,
            tc.tile_pool(name="loss", bufs=1) as loss_pool,
            tc.tile_pool(name="ps", bufs=1, space="PSUM") as psum_pool,
        ):
            # ---- true_sum planes + diag constants (one big DMA each) ----
            tp = tsp_pool.tile([128, N_BITS, BATCH], f8e4)
            nc.sync.dma_start(tp[:], tplanes.rearrange("b p n -> p b n"))
            dg = dg_pool.tile([128, N_BITS * OPC], bf16)
            nc.sync.dma_start(dg[:], diags[:])

            # ---- psum[o, n] = +int_sum (diag matmuls, also warm the PE) --
            psums = [psum_pool.tile([128, NCHUNK], f32, name=f"ps{i}",
                                    tag=f"ps{i}") for i in range(NCH)]
            for b in range(N_BITS):
                for c in range(NCH):
                    nc.tensor.matmul(psums[c][:],
                                     dg[:, b * OPC:(b + 1) * OPC],
                                     tp[:, b, c * NCHUNK:(c + 1) * NCHUNK],
                                     start=(b == 0), stop=False)

            # ---- weight pack (per k-region) + main matmul stream ----
            # acc_g = -int_w for region g's 8 k-tiles; psum -= pred
            accs = [accw_pool.tile([128, RW], bf16, name=f"accw{g}",
                                   tag=f"accw{g}") for g in range(NREG)]
            wps = {}
            for g in range(NREG):
                acc = accs[g]
                if g % 2 == 0:
                    # one ~2MB DMA covers two pack regions (gpsimd/SWDGE
                    # path, concurrent with the latT stream on sync/HWDGE)
                    wp = wp_pool.tile([128, N_BITS, 2 * RW], f8,
                                      name=f"wp{g}", tag="wp")
                    nc.sync.dma_start(
                        wp[:], wplanes.rearrange("b p m -> p b m")[
                            :, :, g * RW:(g + 2) * RW])
                    wps[g] = wp
                    woff = 0
                else:
                    wp = wps[g - 1]
                    woff = RW
                for b in (0, 1, 2, 3, 4, 5, 6, 7):
                    t = wtmp_pool.tile([128, RW], bf16, name=f"t{g}_{b}",
                                       tag="t")
                    nc.scalar.activation(t[:], wp[:, b, woff:woff + RW],
                                         Act.Relu, scale=1e30)
                    if b == 0:
                        nc.vector.tensor_scalar(acc[:], t[:], POWERS[0],
                                                None, Alu.min)
                    elif b < 7:
                        nc.vector.scalar_tensor_tensor(
                            acc[:], t[:], POWERS[b], acc[:],
                            Alu.min, Alu.add)
                    else:
                        nc.vector.scalar_tensor_tensor(
                            acc[:], t[:], 128.0, acc[:],
                            Alu.min, Alu.subtract)
                # 2 quad-k-tile latT DMAs (~1MB) + 8 k-tiles of matmuls
                for kt4 in range(g * KTR // 4, (g + 1) * KTR // 4):
                    lt = lat_pool.tile([128, 4, BATCH], f8e4,
                                       name=f"lt{kt4}", tag="lat")
                    nc.sync.dma_start(
                        lt[:], latT[kt4 * 4 * KP:(kt4 + 1) * 4 * KP, :]
                        .rearrange("(a p) n -> p a n", p=128))
                    for a in range(4):
                        kt = kt4 * 4 + a
                        ktl = kt - g * KTR
                        lhsT = acc[:, ktl * OPC:(ktl + 1) * OPC]
                        for c in range(NCH):
                            nc.tensor.matmul(
                                psums[c][:], lhsT,
                                lt[:, a, c * NCHUNK:(c + 1) * NCHUNK],
                                start=False, stop=(kt == KT - 1))

            # ---- loss: partial[o, c] = sum_n diff^2 (ACT from PSUM) ----
            out_t = loss_pool.tile([128, NCH], f32)
            for c in range(NCH):
                d2 = wtmp_pool.tile([128, NCHUNK], f32, name=f"d2_{c}",
                                    tag="d2")
                nc.scalar.activation(d2[:], psums[c][:], Act.Square,
                                     accum_out=out_t[:, c:c + 1])
            nc.sync.dma_start(partials[:], out_t[:])

    nc.compile()
    return nc


def _get_nc():
    if "nc" not in _CACHE:
        _CACHE["nc"] = _build()
    return _CACHE["nc"]


def make_in_maps(latent: np.ndarray, true_sum: np.ndarray,
                 weight: np.ndarray) -> list:
    bf = ml_dtypes.bfloat16
    f8 = ml_dtypes.float8_e5m2
    f8e4 = ml_dtypes.float8_e4m3fn
    lat_bf = np.ascontiguousarray(latent.astype(f8e4).T)   # [8192, 2048]
    t_bf = true_sum.astype(f8e4)
    diags = np.zeros((OPC, N_BITS * OPC), dtype=np.float32)
    for b in range(N_BITS):
        np.fill_diagonal(diags[:, b * OPC:(b + 1) * OPC], POWERS[b])
    diags = diags.astype(bf)

    in_maps = []
    for c in range(N_CORES):
        W = weight[:, c * OPC * N_BITS:(c + 1) * OPC * N_BITS]
        # [k, ol*8+b] -> [kt, kp, ol, b] -> [b, kp, kt, ol]
        W4 = W.reshape(KT, KP, OPC, N_BITS).transpose(3, 1, 0, 2)
        wpl = np.ascontiguousarray(W4).reshape(
            N_BITS, KP, KT * OPC).astype(f8)
        T = t_bf[:, c * OPC * N_BITS:(c + 1) * OPC * N_BITS]
        # [n, ol*8+b] -> [n, ol, b] -> [b, ol, n]
        T3 = T.reshape(BATCH, OPC, N_BITS).transpose(2, 1, 0)
        tpl = np.ascontiguousarray(T3)
        in_maps.append({"latT": lat_bf, "wplanes": wpl, "tplanes": tpl,
                        "diags": diags})
    return in_maps


def kernel(latent: np.ndarray, true_sum: np.ndarray,
           weight: np.ndarray) -> np.ndarray:
    from concourse.bass_utils import run_bass_kernel_spmd

    nc = _get_nc()
    in_maps = make_in_maps(latent, true_sum, weight)
    res = run_bass_kernel_spmd(nc, in_maps, list(range(N_CORES)))

    total = 0.0
    for c in range(N_CORES):
        total += float(res.results[c]["partials"].astype(np.float64).sum())
    loss = total / (BATCH * OUT_FEATURES) / (SCALE * SCALE)
    return np.array(loss, dtype=np.float32)
